# revision 16
# baseline (speedup 1.0000x reference)
"""AdaIN segment-reduce kernel for 8 TRN2 NeuronCores (Bass/Tile).

Sharding: pure data parallel over (batch, H-half): core c handles sample
c//2, H-rows half c%2 (131072 pixels, [64, 131072] f32). Per-(b,class)
stats need the full sample, so pair cores {2b, 2b+1} AllReduce their
partial (s1, s2) sums (tiny [128,19] tile). Label histograms (cnt/valid)
and the three small outputs depend only on labels + style tables and are
computed host-side.

Device pipeline per core:
  stats pass: stream x, PE-transpose [64,128] blocks to pixel-on-partition,
    square on ACT, one-hot(label) per 128-px chunk on DVE, accumulate
    s1/s2 = x(T).T @ onehot into one PSUM tile over all 1024 chunks.
  combine: AllReduce over the pair; tiny [64,19] DVE/ACT math produces the
    per-(class,channel) affine A = style_std/std_c, B = style_mean - mean_c*A
    (identity for invalid classes); PE-transpose A,B into a [38,128] table.
  apply pass: per 512-px range, broadcast-DMA labels to 19 partitions,
    one-hot on DVE, gather A/B via [19,64]x[19,512] matmuls into PSUM,
    out = x*A_g + B_g with two DVE tensor-tensor ops.
"""

import sys

sys.path.insert(0, "/opt/trn_rl_repo")

from contextlib import ExitStack

import numpy as np

from concourse import bass, bacc, mybir
import concourse.tile as tile
from concourse.masks import make_identity
from concourse.bass_utils import run_bass_kernel_spmd

NUM_CLASSES = 19
EPS = 1e-5
COUNT = 6
B, C, H, W = 4, 64, 512, 512
HW = H * W
NPIX = HW // 2  # pixels per core (half sample)
NCORES = 8
NMEGA = 16  # stats/apply mega tiles of 8192 px
MPIX = 8192
F32 = mybir.dt.float32
F32R = mybir.dt.float32r
I32 = mybir.dt.int32
AF = mybir.ActivationFunctionType
OP = mybir.AluOpType


def _build_nc():
    nc = bacc.Bacc(num_devices=NCORES)
    x = nc.declare_dram_parameter("x", [C, NPIX], F32, isOutput=False)
    lab = nc.declare_dram_parameter("lab", [1, NPIX], I32, isOutput=False)
    iota19 = nc.declare_dram_parameter("iota19", [128, 19], F32, isOutput=False)
    iota38 = nc.declare_dram_parameter("iota38", [64, 1], F32, isOutput=False)
    invc = nc.declare_dram_parameter("invc", [64, 19], F32, isOutput=False)
    bessel = nc.declare_dram_parameter("bessel", [64, 19], F32, isOutput=False)
    validr = nc.declare_dram_parameter("validr", [64, 19], F32, isOutput=False)
    invalr = nc.declare_dram_parameter("invalr", [64, 19], F32, isOutput=False)
    smeanT = nc.declare_dram_parameter("smeanT", [64, 19], F32, isOutput=False)
    sstdT = nc.declare_dram_parameter("sstdT", [64, 19], F32, isOutput=False)
    ident2 = nc.declare_dram_parameter("ident2", [128, 64], F32, isOutput=False)
    ident128 = nc.declare_dram_parameter("ident128", [128, 128], F32R, isOutput=False)
    zeros52 = nc.declare_dram_parameter("zeros52", [52, 128], F32R, isOutput=False)
    out = nc.declare_dram_parameter("out", [C, NPIX], F32, isOutput=True)

    ccin = nc.dram_tensor("ccin", [128, 19], F32)
    ccout = nc.dram_tensor("ccout", [128, 19], F32)

    with tile.TileContext(nc) as tc, ExitStack() as ctx:
        const_p = ctx.enter_context(tc.tile_pool(name="const", bufs=1))
        setup_p = ctx.enter_context(tc.tile_pool(name="setup", bufs=1))
        xm_p = ctx.enter_context(tc.tile_pool(name="xm", bufs=2))
        big_p = ctx.enter_context(tc.tile_pool(name="big", bufs=2))
        oh_p = ctx.enter_context(tc.tile_pool(name="ohp", bufs=4))
        lbp = ctx.enter_context(tc.tile_pool(name="lbp", bufs=2))
        am_p = ctx.enter_context(tc.tile_pool(name="am", bufs=2))
        om_p = ctx.enter_context(tc.tile_pool(name="om", bufs=2))
        tmp_p = ctx.enter_context(tc.tile_pool(name="tmp", bufs=2))
        ps_t = ctx.enter_context(tc.tile_pool(name="pst", bufs=2, space="PSUM"))
        ps_s = ctx.enter_context(tc.tile_pool(name="pss", bufs=1, space="PSUM"))
        ps_a = ctx.enter_context(tc.tile_pool(name="psa", bufs=2, space="PSUM"))
        ps_b = ctx.enter_context(tc.tile_pool(name="psb", bufs=2, space="PSUM"))

        # ---- constants ----
        ident = const_p.tile([128, 64], F32)
        nc.sync.dma_start(out=ident[:], in_=ident2[:])
        identF = const_p.tile([128, 128], F32R)
        nc.sync.dma_start(out=identF[:], in_=ident128[:])
        io19 = const_p.tile([128, 19], F32)
        nc.sync.dma_start(out=io19[:], in_=iota19[:])
        io38 = const_p.tile([64, 1], F32)
        nc.sync.dma_start(out=io38[:], in_=iota38[:])
        t_invc = const_p.tile([64, 19], F32)
        nc.sync.dma_start(out=t_invc[:], in_=invc[:])
        t_bes = const_p.tile([64, 19], F32)
        nc.sync.dma_start(out=t_bes[:], in_=bessel[:])
        t_val = const_p.tile([64, 19], F32)
        nc.sync.dma_start(out=t_val[:], in_=validr[:])
        t_inv = const_p.tile([64, 19], F32)
        nc.sync.dma_start(out=t_inv[:], in_=invalr[:])
        t_smean = const_p.tile([64, 19], F32)
        nc.sync.dma_start(out=t_smean[:], in_=smeanT[:])
        t_sstd = const_p.tile([64, 19], F32)
        nc.sync.dma_start(out=t_sstd[:], in_=sstdT[:])

        # ---- labels: load as [64, 2048], convert to f32, PE-transpose to
        # labT [128, 1024] where column c = 64*j + p holds labels of pixels
        # [p*2048 + 128*j, +128) ----
        lab64i = lbp.tile([64, 2048], I32, tag="lb")
        nc.sync.dma_start(
            out=lab64i[:], in_=lab[:].rearrange("a (p f) -> (a p) f", p=64)
        )
        lab64f = lbp.tile([64, 2048], F32, tag="oh64")
        nc.vector.tensor_copy(out=lab64f[:], in_=lab64i[:])
        labT = setup_p.tile([128, 1024], F32)
        for j in range(16):
            pl = ps_t.tile([128, 64], F32, tag="pt")
            nc.tensor.transpose(
                out=pl[:],
                in_=lab64f[:, 128 * j : 128 * (j + 1)],
                identity=ident[0:64, :],
            )
            nc.scalar.activation(
                out=labT[:, 64 * j : 64 * j + 64], in_=pl[:], func=AF.Copy
            )

        # ---- stats pass: accumulate s1 (rows 0:64) / s2 (rows 64:128) over
        # all 1024 chunks into one PSUM tile. One 2MB DMA per mega tile;
        # psum->sbuf copies and squares on ACT; one-hot build on DVE. ----
        psS = ps_s.tile([128, 19], F32)
        n_chunks = NPIX // 128
        chunk_idx = 0
        for m2 in range(2 * NMEGA):
            xm = xm_p.tile([64, 4096], F32)
            nc.sync.dma_start(out=xm[:], in_=x[:, 4096 * m2 : 4096 * (m2 + 1)])
            for g in range(4):
                ptile = ps_t.tile([128, 512], F32, tag="pt")
                big = big_p.tile([128, 1024], F32)
                sub = []
                for i in range(8):
                    u = 8 * g + i
                    c = 64 * (u % 16) + 2 * m2 + u // 16
                    sub.append(c)
                    nc.tensor.transpose(
                        out=ptile[:, 64 * i : 64 * i + 64],
                        in_=xm[:, 128 * u : 128 * (u + 1)],
                        identity=ident[0:64, :],
                    )
                bigv = big[:].rearrange("p (i f) -> p i f", i=8)
                ptv = ptile[:].rearrange("p (i f) -> p i f", i=8)
                nc.vector.tensor_copy(out=bigv[:, :, 0:64], in_=ptv[:, :, :])
                nc.scalar.activation(
                    out=bigv[:, :, 64:128], in_=bigv[:, :, 0:64], func=AF.Square
                )
                for i in range(8):
                    c = sub[i]
                    oh = oh_p.tile([128, 19], F32, tag="oh")
                    nc.vector.tensor_scalar(
                        out=oh[:],
                        in0=io19[:],
                        scalar1=labT[:, c : c + 1],
                        scalar2=None,
                        op0=OP.is_equal,
                    )
                    nc.tensor.matmul(
                        out=psS[:],
                        lhsT=big[:, 128 * i : 128 * (i + 1)],
                        rhs=oh[:],
                        start=(chunk_idx == 0),
                        stop=(chunk_idx == n_chunks - 1),
                        skip_group_check=True,
                    )
                    chunk_idx += 1

        # ---- pair AllReduce of (s1|s2) ----
        statsS = setup_p.tile([128, 19], F32)
        nc.vector.tensor_copy(out=statsS[:], in_=psS[:])
        nc.sync.dma_start(out=ccin[:], in_=statsS[:])
        nc.gpsimd.collective_compute(
            "AllReduce",
            OP.add,
            replica_groups=[[0, 1], [2, 3], [4, 5], [6, 7]],
            ins=[ccin[:]],
            outs=[ccout[:]],
        )
        statsF = setup_p.tile([128, 19], F32)
        nc.sync.dma_start(out=statsF[:], in_=ccout[:])

        # ---- tiny per-(channel,class) math: A, B [64, 19] ----
        w_mean = setup_p.tile([64, 19], F32)
        nc.vector.tensor_tensor(
            out=w_mean[:], in0=statsF[0:64, :], in1=t_invc[:], op=OP.mult
        )
        w_s2 = setup_p.tile([64, 19], F32)
        nc.scalar.activation(out=w_s2[:], in_=statsF[64:128, :], func=AF.Copy)
        w_ex2 = setup_p.tile([64, 19], F32)
        nc.vector.tensor_tensor(
            out=w_ex2[:], in0=w_s2[:], in1=t_invc[:], op=OP.mult
        )
        w_m2 = setup_p.tile([64, 19], F32)
        nc.vector.tensor_tensor(out=w_m2[:], in0=w_mean[:], in1=w_mean[:], op=OP.mult)
        w_var = setup_p.tile([64, 19], F32)
        nc.vector.tensor_tensor(out=w_var[:], in0=w_ex2[:], in1=w_m2[:], op=OP.subtract)
        w_var2 = setup_p.tile([64, 19], F32)
        nc.vector.tensor_tensor(out=w_var2[:], in0=w_var[:], in1=t_bes[:], op=OP.mult)
        w_var3 = setup_p.tile([64, 19], F32)
        nc.vector.tensor_scalar(
            out=w_var3[:], in0=w_var2[:], scalar1=0.0, scalar2=None, op0=OP.max
        )
        w_std = setup_p.tile([64, 19], F32)
        nc.scalar.activation(out=w_std[:], in_=w_var3[:], func=AF.Sqrt)
        w_std2 = setup_p.tile([64, 19], F32)
        nc.vector.tensor_scalar(
            out=w_std2[:], in0=w_std[:], scalar1=float(EPS), scalar2=None, op0=OP.add
        )
        w_rstd = setup_p.tile([64, 19], F32)
        nc.vector.reciprocal(out=w_rstd[:], in_=w_std2[:])
        w_A0 = setup_p.tile([64, 19], F32)
        nc.vector.tensor_tensor(out=w_A0[:], in0=t_sstd[:], in1=w_rstd[:], op=OP.mult)
        w_Av = setup_p.tile([64, 19], F32)
        nc.vector.tensor_tensor(out=w_Av[:], in0=w_A0[:], in1=t_val[:], op=OP.mult)
        w_A = setup_p.tile([64, 19], F32)
        nc.vector.tensor_tensor(out=w_A[:], in0=w_Av[:], in1=t_inv[:], op=OP.add)
        w_mA = setup_p.tile([64, 19], F32)
        nc.vector.tensor_tensor(out=w_mA[:], in0=w_mean[:], in1=w_A[:], op=OP.mult)
        w_B0 = setup_p.tile([64, 19], F32)
        nc.vector.tensor_tensor(
            out=w_B0[:], in0=t_smean[:], in1=w_mA[:], op=OP.subtract
        )
        w_B = setup_p.tile([64, 19], F32)
        nc.vector.tensor_tensor(out=w_B[:], in0=w_B0[:], in1=t_val[:], op=OP.mult)

        # Block-diagonal gather stationaries [52, 128] (f32r): rows 0:19 x
        # cols 0:64 = A_T (pixel-range 1), rows 32:51 x cols 64:128 = A_T
        # (range 2); zeros elsewhere kill cross terms, so ONE matmul gathers
        # both ranges into a [128, 512] PSUM at dst partition 0.
        ABkA = const_p.tile([52, 128], F32R)
        ABkB = const_p.tile([52, 128], F32R)
        nc.sync.dma_start(out=ABkA[:], in_=zeros52[:])
        nc.sync.dma_start(out=ABkB[:], in_=zeros52[:])
        pA = ps_t.tile([19, 64], F32, tag="pt")
        nc.tensor.transpose(out=pA[:], in_=w_A[:], identity=ident[0:64, :])
        nc.scalar.activation(out=ABkA[0:19, 0:64], in_=pA[:], func=AF.Copy)
        nc.scalar.activation(out=ABkA[32:51, 64:128], in_=pA[:], func=AF.Copy)
        pB = ps_t.tile([19, 64], F32, tag="pt")
        nc.tensor.transpose(out=pB[:], in_=w_B[:], identity=ident[0:64, :])
        nc.scalar.activation(out=ABkB[0:19, 0:64], in_=pB[:], func=AF.Copy)
        nc.scalar.activation(out=ABkB[32:51, 64:128], in_=pB[:], func=AF.Copy)

        # ---- apply pass ----
        # per mega: x stacked [128,4096] (2 DMAs via ACT ring), labels
        # broadcast [19,4096]x2 via gpsimd SWDGE, one is_equal -> oh64,
        # per 512-px group: 4 f32r gather matmuls, DVE mul, ACT copy of B,
        # Pool add, outputs via DVE ring.
        for m in range(NMEGA):
            am = am_p.tile([128, 4096], F32)
            nc.gpsimd.dma_start(
                out=am[0:64, :], in_=x[:, MPIX * m : MPIX * m + 4096]
            )
            nc.gpsimd.dma_start(
                out=am[64:128, :], in_=x[:, MPIX * m + 4096 : MPIX * (m + 1)]
            )
            lb = lbp.tile([64, 4096], I32, tag="lb")
            nc.gpsimd.dma_start(
                out=lb[0:19, :],
                in_=lab[:, MPIX * m : MPIX * m + 4096].to_broadcast((19, 4096)),
            )
            nc.gpsimd.dma_start(
                out=lb[32:51, :],
                in_=lab[:, MPIX * m + 4096 : MPIX * (m + 1)].to_broadcast((19, 4096)),
            )
            oh64 = lbp.tile([64, 4096], F32R, tag="oh64")
            nc.vector.tensor_scalar(
                out=oh64[:],
                in0=lb[:],
                scalar1=io38[:],
                scalar2=None,
                op0=OP.is_equal,
            )
            om = om_p.tile([128, 4096], F32)
            for g in range(8):
                sl = slice(512 * g, 512 * (g + 1))
                psA = ps_a.tile([128, 512], F32)
                psB = ps_b.tile([128, 512], F32)
                nc.tensor.matmul(
                    out=psA[:, :], lhsT=ABkA[0:52, :], rhs=oh64[0:52, sl],
                    start=True, stop=True,
                )
                nc.tensor.matmul(
                    out=psB[:, :], lhsT=ABkB[0:52, :], rhs=oh64[0:52, sl],
                    start=True, stop=False, skip_group_check=True,
                )
                tmp = tmp_p.tile([128, 512], F32R, tag="tmp")
                nc.vector.tensor_tensor(
                    out=tmp[:], in0=am[:, sl], in1=psA[:], op=OP.mult
                )
                nc.tensor.matmul(
                    out=psB[:, :], lhsT=identF[:], rhs=tmp[:],
                    start=False, stop=True, skip_group_check=True,
                )
                nc.scalar.activation(out=om[:, sl], in_=psB[:], func=AF.Copy)
            nc.sync.dma_start(
                out=out[:, MPIX * m : MPIX * m + 4096], in_=om[0:64, :]
            )
            nc.sync.dma_start(
                out=out[:, MPIX * m + 4096 : MPIX * (m + 1)], in_=om[64:128, :]
            )
    nc.finalize()
    return nc


_NC_CACHE = None


def _get_nc():
    global _NC_CACHE
    if _NC_CACHE is None:
        _NC_CACHE = _build_nc()
    return _NC_CACHE


def _make_in_maps(x_content, y_content, style_means, style_stds):
    x_content = np.asarray(x_content, dtype=np.float32)
    y_content = np.asarray(y_content, dtype=np.int32)
    style_means = np.asarray(style_means, dtype=np.float32)
    style_stds = np.asarray(style_stds, dtype=np.float32)

    cnt = np.zeros((B, NUM_CLASSES), dtype=np.int64)
    yflat = y_content.reshape(B, HW)
    for b in range(B):
        cnt[b] = np.bincount(yflat[b], minlength=NUM_CLASSES)
    valid = cnt > COUNT
    invc = (1.0 / np.maximum(cnt, 1)).astype(np.float32)
    bessel = (cnt / np.maximum(cnt - 1, 1)).astype(np.float32)
    validf = valid.astype(np.float32)

    iota19 = np.tile(np.arange(19, dtype=np.float32), (128, 1))
    iota38 = np.full((64, 1), 255.0, dtype=np.float32)
    iota38[0:19, 0] = np.arange(19)
    iota38[32:51, 0] = np.arange(19)
    smeanT = np.ascontiguousarray(style_means.T)
    ident2 = np.concatenate([np.eye(64, dtype=np.float32)] * 2, axis=0)
    ident128f = np.eye(128, dtype=np.float32)
    zeros52f = np.zeros((52, 128), dtype=np.float32)
    sstdT = np.ascontiguousarray(style_stds.T)

    xr = x_content.reshape(B, C, HW)
    in_maps = []
    for core in range(NCORES):
        b, h = core // 2, core % 2
        in_maps.append(
            {
                "x": np.ascontiguousarray(xr[b, :, h * NPIX : (h + 1) * NPIX]),
                "lab": np.ascontiguousarray(
                    yflat[b, None, h * NPIX : (h + 1) * NPIX]
                ),
                "iota19": iota19,
                "iota38": iota38,
                "invc": np.tile(invc[b], (64, 1)),
                "bessel": np.tile(bessel[b], (64, 1)),
                "validr": np.tile(validf[b], (64, 1)),
                "invalr": np.tile(1.0 - validf[b], (64, 1)),
                "smeanT": smeanT,
                "ident2": ident2,
                "ident128": ident128f,
                "zeros52": zeros52f,
                "sstdT": sstdT,
            }
        )
    return in_maps, valid


def kernel(x_content, y_content, style_means, style_stds, trace=False):
    style_means = np.asarray(style_means, dtype=np.float32)
    style_stds = np.asarray(style_stds, dtype=np.float32)
    in_maps, valid = _make_in_maps(x_content, y_content, style_means, style_stds)
    nc = _get_nc()
    res = run_bass_kernel_spmd(nc, in_maps, core_ids=list(range(NCORES)), trace=trace)
    out = np.empty((B, C, HW), dtype=np.float32)
    for core in range(NCORES):
        b, h = core // 2, core % 2
        out[b, :, h * NPIX : (h + 1) * NPIX] = res.results[core]["out"]
    out = out.reshape(B, C, H, W)

    style_means_1dim = valid[:, :, None] * style_means[None]
    style_stds_1dim = valid[:, :, None] * style_stds[None]
    ret = (
        out,
        style_means_1dim.astype(np.float32),
        style_stds_1dim.astype(np.float32),
        valid,
    )
    if trace:
        return ret, res
    return ret


# revision 18
# speedup vs baseline: 1.0527x; 1.0527x over previous
"""AdaIN segment-reduce kernel for 8 TRN2 NeuronCores (Bass/Tile).

Sharding: pure data parallel over (batch, H-half): core c handles sample
c//2, H-rows half c%2 (131072 pixels, [64, 131072] f32). Per-(b,class)
stats need the full sample, so pair cores {2b, 2b+1} AllReduce their
partial (s1, s2) sums (tiny [128,19] tile). Label histograms (cnt/valid)
and the three small outputs depend only on labels + style tables and are
computed host-side.

Device pipeline per core:
  stats pass: stream x, PE-transpose [64,128] blocks to pixel-on-partition,
    square on ACT, one-hot(label) per 128-px chunk on DVE, accumulate
    s1/s2 = x(T).T @ onehot into one PSUM tile over all 1024 chunks.
  combine: AllReduce over the pair; tiny [64,19] DVE/ACT math produces the
    per-(class,channel) affine A = style_std/std_c, B = style_mean - mean_c*A
    (identity for invalid classes); PE-transpose A,B into a [38,128] table.
  apply pass: per 512-px range, broadcast-DMA labels to 19 partitions,
    one-hot on DVE, gather A/B via [19,64]x[19,512] matmuls into PSUM,
    out = x*A_g + B_g with two DVE tensor-tensor ops.
"""

import sys

sys.path.insert(0, "/opt/trn_rl_repo")

from contextlib import ExitStack

import numpy as np

from concourse import bass, bacc, mybir
import concourse.tile as tile
from concourse.masks import make_identity
from concourse.bass_utils import run_bass_kernel_spmd

NUM_CLASSES = 19
EPS = 1e-5
COUNT = 6
B, C, H, W = 4, 64, 512, 512
HW = H * W
NPIX = HW // 2  # pixels per core (half sample)
NCORES = 8
NMEGA = 16  # stats/apply mega tiles of 8192 px
MPIX = 8192
F32 = mybir.dt.float32
F32R = mybir.dt.float32r
BF16 = mybir.dt.bfloat16
I32 = mybir.dt.int32
AF = mybir.ActivationFunctionType
OP = mybir.AluOpType


def _build_nc():
    nc = bacc.Bacc(num_devices=NCORES)
    x = nc.declare_dram_parameter("x", [C, NPIX], F32, isOutput=False)
    lab = nc.declare_dram_parameter("lab", [1, NPIX], I32, isOutput=False)
    iota19 = nc.declare_dram_parameter("iota19", [128, 19], F32, isOutput=False)
    iota38 = nc.declare_dram_parameter("iota38", [64, 1], F32, isOutput=False)
    invc = nc.declare_dram_parameter("invc", [64, 19], F32, isOutput=False)
    bessel = nc.declare_dram_parameter("bessel", [64, 19], F32, isOutput=False)
    validr = nc.declare_dram_parameter("validr", [64, 19], F32, isOutput=False)
    invalr = nc.declare_dram_parameter("invalr", [64, 19], F32, isOutput=False)
    smeanT = nc.declare_dram_parameter("smeanT", [64, 19], F32, isOutput=False)
    sstdT = nc.declare_dram_parameter("sstdT", [64, 19], F32, isOutput=False)
    ident2 = nc.declare_dram_parameter("ident2", [128, 64], F32, isOutput=False)
    ident128 = nc.declare_dram_parameter("ident128", [128, 128], F32R, isOutput=False)
    zeros52 = nc.declare_dram_parameter("zeros52", [52, 128], F32R, isOutput=False)
    ident128b = nc.declare_dram_parameter("ident128b", [128, 128], BF16, isOutput=False)
    out = nc.declare_dram_parameter("out", [C, NPIX], F32, isOutput=True)

    ccin = nc.dram_tensor("ccin", [128, 19], F32)
    ccout = nc.dram_tensor("ccout", [128, 19], F32)

    with tile.TileContext(nc) as tc, ExitStack() as ctx:
        const_p = ctx.enter_context(tc.tile_pool(name="const", bufs=1))
        setup_p = ctx.enter_context(tc.tile_pool(name="setup", bufs=1))
        xm_p = ctx.enter_context(tc.tile_pool(name="xm", bufs=2))
        big_p = ctx.enter_context(tc.tile_pool(name="big", bufs=2))
        oh_p = ctx.enter_context(tc.tile_pool(name="ohp", bufs=4))
        lbp = ctx.enter_context(tc.tile_pool(name="lbp", bufs=2))
        am_p = ctx.enter_context(tc.tile_pool(name="am", bufs=2))
        om_p = ctx.enter_context(tc.tile_pool(name="om", bufs=2))
        tmp_p = ctx.enter_context(tc.tile_pool(name="tmp", bufs=2))
        ps_t = ctx.enter_context(tc.tile_pool(name="pst", bufs=2, space="PSUM"))
        ps_s = ctx.enter_context(tc.tile_pool(name="pss", bufs=1, space="PSUM"))
        ps_a = ctx.enter_context(tc.tile_pool(name="psa", bufs=2, space="PSUM"))
        ps_b = ctx.enter_context(tc.tile_pool(name="psb", bufs=2, space="PSUM"))

        # ---- constants ----
        ident = const_p.tile([128, 64], F32)
        nc.sync.dma_start(out=ident[:], in_=ident2[:])
        identF = const_p.tile([128, 128], F32R)
        nc.sync.dma_start(out=identF[:], in_=ident128[:])
        identB = const_p.tile([128, 128], BF16)
        nc.sync.dma_start(out=identB[:], in_=ident128b[:])
        io19 = const_p.tile([128, 19], F32)
        nc.sync.dma_start(out=io19[:], in_=iota19[:])
        io38 = const_p.tile([64, 1], F32)
        nc.sync.dma_start(out=io38[:], in_=iota38[:])
        t_invc = const_p.tile([64, 19], F32)
        nc.sync.dma_start(out=t_invc[:], in_=invc[:])
        t_bes = const_p.tile([64, 19], F32)
        nc.sync.dma_start(out=t_bes[:], in_=bessel[:])
        t_val = const_p.tile([64, 19], F32)
        nc.sync.dma_start(out=t_val[:], in_=validr[:])
        t_inv = const_p.tile([64, 19], F32)
        nc.sync.dma_start(out=t_inv[:], in_=invalr[:])
        t_smean = const_p.tile([64, 19], F32)
        nc.sync.dma_start(out=t_smean[:], in_=smeanT[:])
        t_sstd = const_p.tile([64, 19], F32)
        nc.sync.dma_start(out=t_sstd[:], in_=sstdT[:])

        # ---- labels: load as [64, 2048], convert to f32, PE-transpose to
        # labT [128, 1024] where column c = 64*j + p holds labels of pixels
        # [p*2048 + 128*j, +128) ----
        lab64i = lbp.tile([64, 2048], I32, tag="lb")
        nc.sync.dma_start(
            out=lab64i[:], in_=lab[:].rearrange("a (p f) -> (a p) f", p=64)
        )
        lab64f = lbp.tile([64, 2048], F32, tag="oh64")
        nc.vector.tensor_copy(out=lab64f[:], in_=lab64i[:])
        labT = setup_p.tile([128, 1024], F32)
        for j in range(16):
            pl = ps_t.tile([128, 64], F32, tag="pt")
            nc.tensor.transpose(
                out=pl[:],
                in_=lab64f[:, 128 * j : 128 * (j + 1)],
                identity=ident[0:64, :],
            )
            nc.scalar.activation(
                out=labT[:, 64 * j : 64 * j + 64], in_=pl[:], func=AF.Copy
            )

        # ---- stats pass: accumulate s1 (rows 0:64) / s2 (rows 64:128) over
        # all 1024 chunks into one PSUM tile. One 2MB DMA per mega tile;
        # psum->sbuf copies and squares on ACT; one-hot build on DVE. ----
        psS = ps_s.tile([128, 19], F32)
        n_chunks = NPIX // 128
        chunk_idx = 0
        for m in range(NMEGA):
            xmb = xm_p.tile([128, 4096], BF16)
            # gpsimd DMA casts f32 -> bf16 in flight; rows 64:128 = +4096 px
            nc.gpsimd.dma_start(
                out=xmb[0:64, :], in_=x[:, MPIX * m : MPIX * m + 4096]
            )
            nc.gpsimd.dma_start(
                out=xmb[64:128, :], in_=x[:, MPIX * m + 4096 : MPIX * (m + 1)]
            )
            for g in range(8):
                ptile = ps_t.tile([128, 512], BF16, tag="pt")
                big = big_p.tile([128, 1024], BF16)
                subs = []
                for k in range(8):
                    t, half = k // 2, k % 2
                    o = MPIX * m + 4096 * half + 512 * g + 128 * t
                    p_row, j = o // 2048, (o % 2048) // 128
                    subs.append(64 * j + p_row)
                for t in range(4):
                    nc.tensor.transpose(
                        out=ptile[:, 128 * t : 128 * (t + 1)],
                        in_=xmb[:, 512 * g + 128 * t : 512 * g + 128 * (t + 1)],
                        identity=identB[:, :],
                    )
                bigv = big[:].rearrange("p (i f) -> p i f", i=8)
                ptv = ptile[:].rearrange("p (i f) -> p i f", i=8)
                nc.vector.tensor_copy(out=bigv[:, :, 0:64], in_=ptv[:, :, :])
                nc.scalar.activation(
                    out=bigv[:, :, 64:128], in_=bigv[:, :, 0:64], func=AF.Square
                )
                for k in range(8):
                    t, half = k // 2, k % 2
                    # chunk k sits at ptile cols [64k, 64k+64): for half=0 it is
                    # out cols 0:64 of transpose t, for half=1 cols 64:128
                    c = subs[k]
                    oh = oh_p.tile([128, 19], BF16, tag="oh")
                    nc.vector.tensor_scalar(
                        out=oh[:],
                        in0=io19[:],
                        scalar1=labT[:, c : c + 1],
                        scalar2=None,
                        op0=OP.is_equal,
                    )
                    nc.tensor.matmul(
                        out=psS[:],
                        lhsT=big[:, 128 * k : 128 * (k + 1)],
                        rhs=oh[:],
                        start=(chunk_idx == 0),
                        stop=(chunk_idx == n_chunks - 1),
                        skip_group_check=True,
                    )
                    chunk_idx += 1

        # ---- pair AllReduce of (s1|s2) ----
        statsS = setup_p.tile([128, 19], F32)
        nc.vector.tensor_copy(out=statsS[:], in_=psS[:])
        nc.sync.dma_start(out=ccin[:], in_=statsS[:])
        nc.gpsimd.collective_compute(
            "AllReduce",
            OP.add,
            replica_groups=[[0, 1], [2, 3], [4, 5], [6, 7]],
            ins=[ccin[:]],
            outs=[ccout[:]],
        )
        statsF = setup_p.tile([128, 19], F32)
        nc.sync.dma_start(out=statsF[:], in_=ccout[:])

        # ---- tiny per-(channel,class) math: A, B [64, 19] ----
        w_mean = setup_p.tile([64, 19], F32)
        nc.vector.tensor_tensor(
            out=w_mean[:], in0=statsF[0:64, :], in1=t_invc[:], op=OP.mult
        )
        w_s2 = setup_p.tile([64, 19], F32)
        nc.scalar.activation(out=w_s2[:], in_=statsF[64:128, :], func=AF.Copy)
        w_ex2 = setup_p.tile([64, 19], F32)
        nc.vector.tensor_tensor(
            out=w_ex2[:], in0=w_s2[:], in1=t_invc[:], op=OP.mult
        )
        w_m2 = setup_p.tile([64, 19], F32)
        nc.vector.tensor_tensor(out=w_m2[:], in0=w_mean[:], in1=w_mean[:], op=OP.mult)
        w_var = setup_p.tile([64, 19], F32)
        nc.vector.tensor_tensor(out=w_var[:], in0=w_ex2[:], in1=w_m2[:], op=OP.subtract)
        w_var2 = setup_p.tile([64, 19], F32)
        nc.vector.tensor_tensor(out=w_var2[:], in0=w_var[:], in1=t_bes[:], op=OP.mult)
        w_var3 = setup_p.tile([64, 19], F32)
        nc.vector.tensor_scalar(
            out=w_var3[:], in0=w_var2[:], scalar1=0.0, scalar2=None, op0=OP.max
        )
        w_std = setup_p.tile([64, 19], F32)
        nc.scalar.activation(out=w_std[:], in_=w_var3[:], func=AF.Sqrt)
        w_std2 = setup_p.tile([64, 19], F32)
        nc.vector.tensor_scalar(
            out=w_std2[:], in0=w_std[:], scalar1=float(EPS), scalar2=None, op0=OP.add
        )
        w_rstd = setup_p.tile([64, 19], F32)
        nc.vector.reciprocal(out=w_rstd[:], in_=w_std2[:])
        w_A0 = setup_p.tile([64, 19], F32)
        nc.vector.tensor_tensor(out=w_A0[:], in0=t_sstd[:], in1=w_rstd[:], op=OP.mult)
        w_Av = setup_p.tile([64, 19], F32)
        nc.vector.tensor_tensor(out=w_Av[:], in0=w_A0[:], in1=t_val[:], op=OP.mult)
        w_A = setup_p.tile([64, 19], F32)
        nc.vector.tensor_tensor(out=w_A[:], in0=w_Av[:], in1=t_inv[:], op=OP.add)
        w_mA = setup_p.tile([64, 19], F32)
        nc.vector.tensor_tensor(out=w_mA[:], in0=w_mean[:], in1=w_A[:], op=OP.mult)
        w_B0 = setup_p.tile([64, 19], F32)
        nc.vector.tensor_tensor(
            out=w_B0[:], in0=t_smean[:], in1=w_mA[:], op=OP.subtract
        )
        w_B = setup_p.tile([64, 19], F32)
        nc.vector.tensor_tensor(out=w_B[:], in0=w_B0[:], in1=t_val[:], op=OP.mult)

        # Block-diagonal gather stationaries [52, 128] (f32r): rows 0:19 x
        # cols 0:64 = A_T (pixel-range 1), rows 32:51 x cols 64:128 = A_T
        # (range 2); zeros elsewhere kill cross terms, so ONE matmul gathers
        # both ranges into a [128, 512] PSUM at dst partition 0.
        ABkA = const_p.tile([52, 128], F32R)
        ABkB = const_p.tile([52, 128], F32R)
        nc.sync.dma_start(out=ABkA[:], in_=zeros52[:])
        nc.sync.dma_start(out=ABkB[:], in_=zeros52[:])
        pA = ps_t.tile([19, 64], F32, tag="pt")
        nc.tensor.transpose(out=pA[:], in_=w_A[:], identity=ident[0:64, :])
        nc.scalar.activation(out=ABkA[0:19, 0:64], in_=pA[:], func=AF.Copy)
        nc.scalar.activation(out=ABkA[32:51, 64:128], in_=pA[:], func=AF.Copy)
        pB = ps_t.tile([19, 64], F32, tag="pt")
        nc.tensor.transpose(out=pB[:], in_=w_B[:], identity=ident[0:64, :])
        nc.scalar.activation(out=ABkB[0:19, 0:64], in_=pB[:], func=AF.Copy)
        nc.scalar.activation(out=ABkB[32:51, 64:128], in_=pB[:], func=AF.Copy)

        # ---- apply pass ----
        # per mega: x stacked [128,4096] (2 DMAs via ACT ring), labels
        # broadcast [19,4096]x2 via gpsimd SWDGE, one is_equal -> oh64,
        # per 512-px group: 4 f32r gather matmuls, DVE mul, ACT copy of B,
        # Pool add, outputs via DVE ring.
        for m in range(NMEGA):
            am = am_p.tile([128, 4096], F32)
            nc.gpsimd.dma_start(
                out=am[0:64, :], in_=x[:, MPIX * m : MPIX * m + 4096]
            )
            nc.gpsimd.dma_start(
                out=am[64:128, :], in_=x[:, MPIX * m + 4096 : MPIX * (m + 1)]
            )
            lb = lbp.tile([64, 4096], I32, tag="lb")
            nc.gpsimd.dma_start(
                out=lb[0:19, :],
                in_=lab[:, MPIX * m : MPIX * m + 4096].to_broadcast((19, 4096)),
            )
            nc.gpsimd.dma_start(
                out=lb[32:51, :],
                in_=lab[:, MPIX * m + 4096 : MPIX * (m + 1)].to_broadcast((19, 4096)),
            )
            oh64 = lbp.tile([64, 4096], F32R, tag="oh64")
            nc.vector.tensor_scalar(
                out=oh64[:],
                in0=lb[:],
                scalar1=io38[:],
                scalar2=None,
                op0=OP.is_equal,
            )
            om = om_p.tile([128, 4096], F32)
            for g in range(8):
                sl = slice(512 * g, 512 * (g + 1))
                psA = ps_a.tile([128, 512], F32)
                psB = ps_b.tile([128, 512], F32)
                nc.tensor.matmul(
                    out=psA[:, :], lhsT=ABkA[0:52, :], rhs=oh64[0:52, sl],
                    start=True, stop=True,
                )
                nc.tensor.matmul(
                    out=psB[:, :], lhsT=ABkB[0:52, :], rhs=oh64[0:52, sl],
                    start=True, stop=False, skip_group_check=True,
                )
                tmp = tmp_p.tile([128, 512], F32R, tag="tmp")
                nc.vector.tensor_tensor(
                    out=tmp[:], in0=am[:, sl], in1=psA[:], op=OP.mult
                )
                nc.tensor.matmul(
                    out=psB[:, :], lhsT=identF[:], rhs=tmp[:],
                    start=False, stop=True, skip_group_check=True,
                )
                nc.scalar.activation(out=om[:, sl], in_=psB[:], func=AF.Copy)
            nc.sync.dma_start(
                out=out[:, MPIX * m : MPIX * m + 4096], in_=om[0:64, :]
            )
            nc.sync.dma_start(
                out=out[:, MPIX * m + 4096 : MPIX * (m + 1)], in_=om[64:128, :]
            )
    nc.finalize()
    return nc


_NC_CACHE = None


def _get_nc():
    global _NC_CACHE
    if _NC_CACHE is None:
        _NC_CACHE = _build_nc()
    return _NC_CACHE


def _make_in_maps(x_content, y_content, style_means, style_stds):
    x_content = np.asarray(x_content, dtype=np.float32)
    y_content = np.asarray(y_content, dtype=np.int32)
    style_means = np.asarray(style_means, dtype=np.float32)
    style_stds = np.asarray(style_stds, dtype=np.float32)

    cnt = np.zeros((B, NUM_CLASSES), dtype=np.int64)
    yflat = y_content.reshape(B, HW)
    for b in range(B):
        cnt[b] = np.bincount(yflat[b], minlength=NUM_CLASSES)
    valid = cnt > COUNT
    invc = (1.0 / np.maximum(cnt, 1)).astype(np.float32)
    bessel = (cnt / np.maximum(cnt - 1, 1)).astype(np.float32)
    validf = valid.astype(np.float32)

    iota19 = np.tile(np.arange(19, dtype=np.float32), (128, 1))
    iota38 = np.full((64, 1), 255.0, dtype=np.float32)
    iota38[0:19, 0] = np.arange(19)
    iota38[32:51, 0] = np.arange(19)
    smeanT = np.ascontiguousarray(style_means.T)
    ident2 = np.concatenate([np.eye(64, dtype=np.float32)] * 2, axis=0)
    ident128f = np.eye(128, dtype=np.float32)
    zeros52f = np.zeros((52, 128), dtype=np.float32)
    import ml_dtypes
    ident128b = np.eye(128).astype(ml_dtypes.bfloat16)
    sstdT = np.ascontiguousarray(style_stds.T)

    xr = x_content.reshape(B, C, HW)
    in_maps = []
    for core in range(NCORES):
        b, h = core // 2, core % 2
        in_maps.append(
            {
                "x": np.ascontiguousarray(xr[b, :, h * NPIX : (h + 1) * NPIX]),
                "lab": np.ascontiguousarray(
                    yflat[b, None, h * NPIX : (h + 1) * NPIX]
                ),
                "iota19": iota19,
                "iota38": iota38,
                "invc": np.tile(invc[b], (64, 1)),
                "bessel": np.tile(bessel[b], (64, 1)),
                "validr": np.tile(validf[b], (64, 1)),
                "invalr": np.tile(1.0 - validf[b], (64, 1)),
                "smeanT": smeanT,
                "ident2": ident2,
                "ident128": ident128f,
                "zeros52": zeros52f,
                "ident128b": ident128b,
                "sstdT": sstdT,
            }
        )
    return in_maps, valid


def kernel(x_content, y_content, style_means, style_stds, trace=False):
    style_means = np.asarray(style_means, dtype=np.float32)
    style_stds = np.asarray(style_stds, dtype=np.float32)
    in_maps, valid = _make_in_maps(x_content, y_content, style_means, style_stds)
    nc = _get_nc()
    res = run_bass_kernel_spmd(nc, in_maps, core_ids=list(range(NCORES)), trace=trace)
    out = np.empty((B, C, HW), dtype=np.float32)
    for core in range(NCORES):
        b, h = core // 2, core % 2
        out[b, :, h * NPIX : (h + 1) * NPIX] = res.results[core]["out"]
    out = out.reshape(B, C, H, W)

    style_means_1dim = valid[:, :, None] * style_means[None]
    style_stds_1dim = valid[:, :, None] * style_stds[None]
    ret = (
        out,
        style_means_1dim.astype(np.float32),
        style_stds_1dim.astype(np.float32),
        valid,
    )
    if trace:
        return ret, res
    return ret
